# revision 14
# baseline (speedup 1.0000x reference)
"""Trainium2 Bass kernel for the vq_codebook problem.

  dist_sq[n,k] = sum_d (x[n,d]-ctrs[k,d])^2 * s[d]
  out = softmax(-dist_sq, axis=1) @ values

Sharding: data-parallel over N (8192 rows of x per core); ctrs/values/s
replicated on all 8 cores. No collectives (forward only).

Math trick: softmax is shift-invariant, so
  softmax(-dist_sq)[n,k] = softmax(2*cross_s[n,k] - c_sq[k])  with
  cross_s = (x*s) @ ctrs.T,  c_sq[k] = sum_d s[d]*ctrs[k,d]^2.
We compute E = exp(2*cross_s - c_sq) unnormalized (range-checked: max
exponent ~48 < 88, row-max min ~ -27, so fp32 exp never overflows and
denominators stay normal), then
  y[n,:] = (E.T @ values_aug)[n,:256] / (E.T @ values_aug)[n,256]
with values_aug = [values | ones] so the denominator comes from the same
accumulating matmul.

Layouts: phase 1 runs transposed (k on partitions, n on free): per
128-centroid chunk, stationary lhs1 = fp16(s*ctrs^T) [64, 128] against
the moving fp16 x^T [64, 512]. The -c_sq shift is applied as the exp
activation's per-partition f32 bias (partitions = k in this layout), so
it costs nothing on the PE and carries no fp16 quantization error. To
share one bias per activation, each exp covers the same chunk of TWO
row-tiles ([128, 2, 512]); the two matmuls of such a pair also share
the same stationary, halving weight loads. Phase-1 fp16 operands add
~4e-3 rel err vs the 2e-2 budget (validated numerically); fp16 gets PE
fast-weight-load so LDWEIGHTS hides under the previous matmul stream.

x^T and ctrs^T are produced by the DMA XBAR transpose engine (2-byte
dtypes), keeping the PE entirely free of transpose work. The XBAR
maps in[128, (blk 128)] -> out[d, blk, p] = in[p, blk*128 + d] (middle
output dim strides the source free index by 128 = the XBAR tile width
— verified against hardware), so x is staged in a padded [*, 128]
free layout with data in cols 0..63; the junk columns transpose into
partitions 64..127, which no access pattern ever reads.

Phase 2 uses bf16 E chunks as the stationary operand against
values_aug, producing y in natural [n, d_out] layout (fp8/DoubleRow was
evaluated numerically and busts the error budget; bf16 is the floor).

Pipelining: engine queues are in-order, so phase-1 chunk-pairs and
phase-2 sub-tiles are interleaved 1:1 in emission order — while the ACT
engine drains an exp (~1us), the PE streams the previous pair's phase-2
matmuls instead of stalling on the psA pool. The lhs1/c_sq prep (DVE +
ACT only) is likewise interleaved with pair-0 chunk by chunk.
"""

import os

os.environ.setdefault("JAX_PLATFORMS", "axon")

import numpy as np

N, D_IN, K, D_OUT = 65536, 64, 1024, 256
NCORES = 8
NS = N // NCORES  # 8192 rows per core
TROWS = 512  # rows of x per tile
NTILES = NS // TROWS  # 16
NPAIRS = NTILES // 2  # tiles are processed in pairs sharing exp bias
KC = K // 128  # 8 centroid chunks
NSUB = TROWS // 128  # 4 output sub-tiles per tile

USE_F32R = True

_cache = {}


def _build(rows=NS):
    import concourse.bacc as bacc
    import concourse.tile as tile
    from concourse import mybir

    f32 = mybir.dt.float32
    f16 = mybir.dt.float16
    bf16 = mybir.dt.bfloat16
    Exp = mybir.ActivationFunctionType.Exp
    Copy = mybir.ActivationFunctionType.Copy
    Mult = mybir.AluOpType.mult
    Add = mybir.AluOpType.add

    ntiles = rows // TROWS
    npairs = ntiles // 2
    nc = bacc.Bacc("TRN2", target_bir_lowering=False, debug=False)
    x = nc.declare_dram_parameter("x", [rows, D_IN], f32, isOutput=False)
    ctrs = nc.declare_dram_parameter("ctrs", [K, D_IN], f32, isOutput=False)
    values = nc.declare_dram_parameter("values", [K, D_OUT], f32, isOutput=False)
    s = nc.declare_dram_parameter("s", [D_IN], f32, isOutput=False)
    y = nc.declare_dram_parameter("y", [rows, D_OUT], f32, isOutput=True)

    with tile.TileContext(nc) as tc:
        with (
            tc.tile_pool(name="const", bufs=1) as constp,
            tc.tile_pool(name="tmp1", bufs=2) as tmp1p,
            tc.tile_pool(name="xt32", bufs=4) as xt32p,
            tc.tile_pool(name="xsT", bufs=4) as xsTp,
            tc.tile_pool(name="E", bufs=2) as Ep,
            tc.tile_pool(name="ysb", bufs=3) as yp,
            tc.tile_pool(name="rcp", bufs=8) as rcpp,
            tc.tile_pool(name="psA", bufs=3, space="PSUM") as psA,
            tc.tile_pool(name="psO", bufs=2, space="PSUM") as psO,
        ):
            # ---- startup DMAs first: tile-0/1 x, then the small constants --
            def issue_x_dma(i):
                n0 = i * TROWS
                xt32 = xt32p.tile([128, NSUB, D_IN], f32)
                # gpsimd swdge ring: keeps the sync ring free for the XBAR
                # transposes (which gate each pair's phase 1) and the scalar
                # queue free for the exp stream
                nc.gpsimd.dma_start(
                    xt32[:], x[n0 : n0 + TROWS, :].rearrange("(a p) d -> p a d", p=128)
                )
                return xt32

            xt_inflight = [issue_x_dma(0), issue_x_dma(1)]

            s_col = constp.tile([D_IN, 1], f32)
            nc.sync.dma_start(s_col[:], s[:].rearrange("(p o) -> p o", o=1))
            s_row = constp.tile([1, D_IN], f32)
            nc.sync.dma_start(s_row[:], s[:].rearrange("(o d) -> o d", o=1))
            ctrs32 = constp.tile([128, KC, D_IN], f32)
            nc.sync.dma_start(
                ctrs32[:], ctrs[:].rearrange("(c p) d -> p c d", p=128)
            )

            # s broadcast along partitions via a 1-deep outer-product matmul
            ones1 = constp.tile([1, 128], f32)
            nc.vector.memset(ones1[:], 1.0)
            sbc_ps = psO.tile([128, D_OUT + 2], f32, tag="psO")
            nc.tensor.matmul(sbc_ps[:, 0:D_IN], ones1[:], s_row[:])
            sbc = constp.tile([128, D_IN], f32)
            nc.vector.tensor_copy(sbc[:], sbc_ps[:, 0:D_IN])

            # ctrs^T via fp16 cast + XBAR transpose (padded free layout)
            ctrs16 = constp.tile([128, KC, 128], f16)
            nc.vector.memset(ctrs16[:, :, D_IN:128], 0.0)
            nc.vector.tensor_copy(ctrs16[:, :, 0:D_IN], ctrs32[:])
            ctrsT = constp.tile([128, KC, 128], f16)
            nc.sync.dma_start_transpose(
                ctrsT[:], ctrs16[:].rearrange("p c d -> p (c d)")
            )

            # lhs1[d, c, k'] = fp16(s[d] * ctrs[c*128+k', d]); negcsq[k', c]
            # = -sum_d s[d]*ctrs[c*128+k', d]^2 stays f32 (exp bias).
            lhs1 = constp.tile([D_IN, KC, 128], f16)
            negcsq = constp.tile([128, KC], f32)

            def prep_chunk(c):
                nc.scalar.activation(
                    lhs1[:, c, :], ctrsT[0:D_IN, c, :], Copy, scale=s_col[:]
                )
                t1 = tmp1p.tile([128, D_IN], f32)
                nc.vector.tensor_mul(t1[:], ctrs32[:, c, :], sbc[:])
                t2 = tmp1p.tile([128, D_IN], f32)
                nc.vector.tensor_mul(t2[:], t1[:], ctrs32[:, c, :])
                # (tensor_tensor_reduce w/ accum_out wedges the exec unit on
                # HW; tensor_reduce is fine)
                nc.vector.tensor_reduce(
                    negcsq[:, c : c + 1], t2[:],
                    axis=mybir.AxisListType.X, op=Add, negate=True,
                )

            # values staging is only needed once phase 2 of pair 0 starts;
            # emitted inside the loop after pair-1's x loads so it doesn't
            # delay them on the gpsimd ring
            vals_stage = constp.tile([128, KC, D_OUT], f32)
            ones_kc = constp.tile([128, KC, 2], f32)
            vals = constp.tile([128, KC, D_OUT + 2], bf16)

            def stage_vals():
                nc.gpsimd.dma_start(
                    vals_stage[:], values[:].rearrange("(c p) v -> p c v", p=128)
                )
                nc.vector.memset(ones_kc[:], 1.0)
                nc.vector.tensor_copy(vals[:, :, 0:D_OUT], vals_stage[:])
                nc.vector.tensor_copy(vals[:, :, D_OUT : D_OUT + 2], ones_kc[:])

            # ---------- per-tile x pipeline: cast + XBAR transpose ----------
            # Persistent double-buffered fp16 staging so the pad columns
            # (64..127, junk after transpose) are zeroed exactly once.
            xt16_all = constp.tile([128, 2, NSUB, 128], f16)
            nc.vector.memset(xt16_all[:, :, :, D_IN:128], 0.0)

            def assemble_xsT(xt32, i):
                slot = i % 2
                nc.vector.tensor_copy(xt16_all[:, slot, :, 0:D_IN], xt32[:])
                xsT = xsTp.tile([128, NSUB, 128], f16)
                # out[d, a, p] = in[p, a*128 + d]  (hardware XBAR mapping);
                # x lives in cols 0..63 of each 128 block -> rows 0..63.
                nc.sync.dma_start_transpose(
                    xsT[:], xt16_all[:, slot, :, :].rearrange("p a d -> p (a d)")
                )
                return xsT

            # ---------------- main loop ----------------
            def phase1_chunk(xsT2, E2, c):
                # same chunk of two row-tiles: shared stationary + shared
                # f32 c_sq bias on the single exp
                pe = psA.tile([128, 2, TROWS], f32, tag="psA")
                nc.tensor.matmul(pe[:, 0, :], lhs1[:, c, :], xsT2[0][0:D_IN, :, :])
                nc.tensor.matmul(pe[:, 1, :], lhs1[:, c, :], xsT2[1][0:D_IN, :, :])
                nc.scalar.activation(
                    E2[:, c, :, :], pe[:], Exp, scale=2.0,
                    bias=negcsq[:, c : c + 1],
                )

            def phase2_subtile(E2, t, gi, ysb, a):
                po = psO.tile([128, D_OUT + 2], f32, tag="psO")
                for c in range(KC):
                    nc.tensor.matmul(
                        po[:],
                        E2[:, c, t, a * 128 : (a + 1) * 128],
                        vals[:, c, :],
                        start=(c == 0),
                        stop=(c == KC - 1),
                    )
                rcp = rcpp.tile([128, 1], f32)
                nc.vector.reciprocal(rcp[:], po[:, D_OUT : D_OUT + 1])
                nc.vector.tensor_scalar_mul(ysb[:, a, :], po[:, 0:D_OUT], rcp[:])
                if a == NSUB - 1:
                    n0 = gi * TROWS
                    nc.gpsimd.dma_start(
                        y[n0 : n0 + TROWS, :].rearrange("(a p) v -> p a v", p=128),
                        ysb[:],
                    )

            xsT_cur = [
                assemble_xsT(xt_inflight[0], 0),
                assemble_xsT(xt_inflight[1], 1),
            ]
            Eprev = None
            ysb_pair = [None, None]
            for pi in range(npairs):
                if pi + 1 < npairs:
                    xt_inflight = [
                        issue_x_dma(2 * pi + 2),
                        issue_x_dma(2 * pi + 3),
                    ]
                if pi == 0:
                    stage_vals()
                E2 = Ep.tile([128, KC, 2, TROWS], bf16)
                xsT_next = None
                # 8 phase-1 chunk-pairs interleave 1:1 with the previous
                # pair's 8 phase-2 sub-tiles (engine queues are in-order:
                # the PE streams phase-2 while ACT drains the exp).
                for c in range(KC):
                    if pi == 0:
                        prep_chunk(c)
                    phase1_chunk(xsT_cur, E2, c)
                    if c == 2 and pi + 1 < npairs:
                        xsT_next = [
                            assemble_xsT(xt_inflight[0], 2 * pi + 2),
                            assemble_xsT(xt_inflight[1], 2 * pi + 3),
                        ]
                    if Eprev is not None:
                        t, a = divmod(c, NSUB)
                        if a == 0:
                            ysb_pair[t] = yp.tile(
                                [128, NSUB, D_OUT], f32, name="ysb"
                            )
                        phase2_subtile(
                            Eprev, t, 2 * (pi - 1) + t, ysb_pair[t], a
                        )
                Eprev = E2
                if xsT_next is not None:
                    xsT_cur = xsT_next
            for idx in range(2 * NSUB):
                t, a = divmod(idx, NSUB)
                if a == 0:
                    ysb_pair[t] = yp.tile([128, NSUB, D_OUT], f32, name="ysb")
                phase2_subtile(Eprev, t, 2 * (npairs - 1) + t, ysb_pair[t], a)

    nc.compile()
    nc.finalize()
    return nc


def get_nc(use_f32r=USE_F32R, rows=NS, dma="sync", ph2_bf16=True):
    key = ("nc", rows)
    if key not in _cache:
        _cache[key] = _build(rows)
    return _cache[key]


def make_in_maps(x, ctrs, values, s):
    x = np.ascontiguousarray(x, dtype=np.float32)
    ctrs = np.ascontiguousarray(ctrs, dtype=np.float32)
    values = np.ascontiguousarray(values, dtype=np.float32)
    s = np.ascontiguousarray(s, dtype=np.float32)
    return [
        {
            "x": x[i * NS : (i + 1) * NS],
            "ctrs": ctrs,
            "values": values,
            "s": s,
        }
        for i in range(NCORES)
    ]


def run(x, ctrs, values, s, trace=False, use_f32r=USE_F32R, tmpdir=None):
    from concourse.bass_utils import run_bass_kernel_spmd

    nc = get_nc(use_f32r)
    res = run_bass_kernel_spmd(
        nc,
        make_in_maps(x, ctrs, values, s),
        list(range(NCORES)),
        trace=trace,
        tmpdir=tmpdir,
    )
    out = np.concatenate([res.results[i]["y"] for i in range(NCORES)], axis=0)
    return out, res


def kernel(x, ctrs, values, s):
    out, _ = run(x, ctrs, values, s, trace=False)
    return out.astype(np.float32)


# revision 15
# speedup vs baseline: 1.0230x; 1.0230x over previous
"""Trainium2 Bass kernel for the vq_codebook problem.

  dist_sq[n,k] = sum_d (x[n,d]-ctrs[k,d])^2 * s[d]
  out = softmax(-dist_sq, axis=1) @ values

Sharding: data-parallel over N (8192 rows of x per core); ctrs/values/s
replicated on all 8 cores. No collectives (forward only).

Math trick: softmax is shift-invariant, so
  softmax(-dist_sq)[n,k] = softmax(2*cross_s[n,k] - c_sq[k])  with
  cross_s = (x*s) @ ctrs.T,  c_sq[k] = sum_d s[d]*ctrs[k,d]^2.
We compute E = exp(2*cross_s - c_sq) unnormalized (range-checked: max
exponent ~48 < 88, row-max min ~ -27, so fp32 exp never overflows and
denominators stay normal), then
  y[n,:] = (E.T @ values_aug)[n,:256] / (E.T @ values_aug)[n,256]
with values_aug = [values | ones] so the denominator comes from the same
accumulating matmul.

Layouts: phase 1 runs transposed (k on partitions, n on free): per
128-centroid chunk, stationary lhs1 = fp16(s*ctrs^T) [64, 128] against
the moving fp16 x^T [64, 512]. The -c_sq shift is applied as the exp
activation's per-partition f32 bias (partitions = k in this layout), so
it costs nothing on the PE and carries no fp16 quantization error. To
share one bias per activation, each exp covers the same chunk of TWO
row-tiles ([128, 2, 512]); the two matmuls of such a pair also share
the same stationary, halving weight loads. Phase-1 fp16 operands add
~4e-3 rel err vs the 2e-2 budget (validated numerically); fp16 gets PE
fast-weight-load so LDWEIGHTS hides under the previous matmul stream.

x^T and ctrs^T are produced by the DMA XBAR transpose engine (2-byte
dtypes), keeping the PE entirely free of transpose work. The XBAR
maps in[128, (blk 128)] -> out[d, blk, p] = in[p, blk*128 + d] (middle
output dim strides the source free index by 128 = the XBAR tile width
— verified against hardware), so x is staged in a padded [*, 128]
free layout with data in cols 0..63; the junk columns transpose into
partitions 64..127, which no access pattern ever reads.

Phase 2 uses bf16 E chunks as the stationary operand against
values_aug, producing y in natural [n, d_out] layout (fp8/DoubleRow was
evaluated numerically and busts the error budget; bf16 is the floor).

Pipelining: engine queues are in-order, so phase-1 chunk-pairs and
phase-2 sub-tiles are interleaved 1:1 in emission order — while the ACT
engine drains an exp (~1us), the PE streams the previous pair's phase-2
matmuls instead of stalling on the psA pool. The lhs1/c_sq prep (DVE +
ACT only) is likewise interleaved with pair-0 chunk by chunk.
"""

import os

os.environ.setdefault("JAX_PLATFORMS", "axon")

import numpy as np

N, D_IN, K, D_OUT = 65536, 64, 1024, 256
NCORES = 8
NS = N // NCORES  # 8192 rows per core
TROWS = 512  # rows of x per tile
NTILES = NS // TROWS  # 16
NPAIRS = NTILES // 2  # tiles are processed in pairs sharing exp bias
KC = K // 128  # 8 centroid chunks
NSUB = TROWS // 128  # 4 output sub-tiles per tile

USE_F32R = True

_cache = {}


def _build(rows=NS):
    import concourse.bacc as bacc
    import concourse.tile as tile
    from concourse import mybir

    f32 = mybir.dt.float32
    f16 = mybir.dt.float16
    bf16 = mybir.dt.bfloat16
    Exp = mybir.ActivationFunctionType.Exp
    Copy = mybir.ActivationFunctionType.Copy
    Mult = mybir.AluOpType.mult
    Add = mybir.AluOpType.add

    ntiles = rows // TROWS
    npairs = ntiles // 2
    nc = bacc.Bacc("TRN2", target_bir_lowering=False, debug=False)
    x = nc.declare_dram_parameter("x", [rows, D_IN], f32, isOutput=False)
    ctrs = nc.declare_dram_parameter("ctrs", [K, D_IN], f32, isOutput=False)
    values = nc.declare_dram_parameter("values", [K, D_OUT], f32, isOutput=False)
    s = nc.declare_dram_parameter("s", [D_IN], f32, isOutput=False)
    y = nc.declare_dram_parameter("y", [rows, D_OUT], f32, isOutput=True)

    with tile.TileContext(nc) as tc:
        with (
            tc.tile_pool(name="const", bufs=1) as constp,
            tc.tile_pool(name="tmp1", bufs=2) as tmp1p,
            tc.tile_pool(name="xt32", bufs=4) as xt32p,
            tc.tile_pool(name="xsT", bufs=4) as xsTp,
            tc.tile_pool(name="E", bufs=2) as Ep,
            tc.tile_pool(name="ysb", bufs=3) as yp,
            tc.tile_pool(name="rcp", bufs=8) as rcpp,
            tc.tile_pool(name="psA", bufs=3, space="PSUM") as psA,
            tc.tile_pool(name="psO", bufs=2, space="PSUM") as psO,
        ):
            # ---- startup DMAs first: tile-0/1 x, then the small constants --
            def issue_x_dma(i):
                n0 = i * TROWS
                xt32 = xt32p.tile([128, NSUB, D_IN], f32)
                nc.sync.dma_start(
                    xt32[:], x[n0 : n0 + TROWS, :].rearrange("(a p) d -> p a d", p=128)
                )
                return xt32

            xt_inflight = [issue_x_dma(0), issue_x_dma(1)]

            s_col = constp.tile([D_IN, 1], f32)
            nc.sync.dma_start(s_col[:], s[:].rearrange("(p o) -> p o", o=1))
            s_row = constp.tile([1, D_IN], f32)
            nc.sync.dma_start(s_row[:], s[:].rearrange("(o d) -> o d", o=1))
            ctrs32 = constp.tile([128, KC, D_IN], f32)
            nc.sync.dma_start(
                ctrs32[:], ctrs[:].rearrange("(c p) d -> p c d", p=128)
            )

            # s broadcast along partitions via a 1-deep outer-product matmul
            ones1 = constp.tile([1, 128], f32)
            nc.vector.memset(ones1[:], 1.0)
            sbc_ps = psO.tile([128, D_OUT + 2], f32, tag="psO")
            nc.tensor.matmul(sbc_ps[:, 0:D_IN], ones1[:], s_row[:])
            sbc = constp.tile([128, D_IN], f32)
            nc.vector.tensor_copy(sbc[:], sbc_ps[:, 0:D_IN])

            # ctrs^T via fp16 cast + XBAR transpose (padded free layout)
            ctrs16 = constp.tile([128, KC, 128], f16)
            nc.vector.memset(ctrs16[:, :, D_IN:128], 0.0)
            nc.vector.tensor_copy(ctrs16[:, :, 0:D_IN], ctrs32[:])
            ctrsT = constp.tile([128, KC, 128], f16)
            nc.sync.dma_start_transpose(
                ctrsT[:], ctrs16[:].rearrange("p c d -> p (c d)")
            )

            # lhs1[d, c, k'] = fp16(s[d] * ctrs[c*128+k', d]); negcsq[k', c]
            # = -sum_d s[d]*ctrs[c*128+k', d]^2 stays f32 (exp bias).
            lhs1 = constp.tile([D_IN, KC, 128], f16)
            negcsq = constp.tile([128, KC], f32)

            def prep_chunk(c):
                # DVE per-partition scalar multiply: keeps the ACT engine
                # pure-Exp (an ACT Copy here would thrash the 1.5us
                # activation table load between Copy and Exp sets)
                nc.vector.tensor_scalar_mul(
                    lhs1[:, c, :], ctrsT[0:D_IN, c, :], s_col[:]
                )
                t1 = tmp1p.tile([128, D_IN], f32)
                nc.vector.tensor_mul(t1[:], ctrs32[:, c, :], sbc[:])
                t2 = tmp1p.tile([128, D_IN], f32)
                nc.vector.tensor_mul(t2[:], t1[:], ctrs32[:, c, :])
                # (tensor_tensor_reduce w/ accum_out wedges the exec unit on
                # HW; tensor_reduce is fine)
                nc.vector.tensor_reduce(
                    negcsq[:, c : c + 1], t2[:],
                    axis=mybir.AxisListType.X, op=Add, negate=True,
                )

            # values staging is only needed once phase 2 of pair 0 starts;
            # emitted inside the loop after pair-1's x loads so it doesn't
            # delay them on the gpsimd ring
            vals_stage = constp.tile([128, KC, D_OUT], f32)
            ones_kc = constp.tile([128, KC, 2], f32)
            vals = constp.tile([128, KC, D_OUT + 2], bf16)

            def stage_vals():
                nc.gpsimd.dma_start(
                    vals_stage[:], values[:].rearrange("(c p) v -> p c v", p=128)
                )
                nc.vector.memset(ones_kc[:], 1.0)
                nc.vector.tensor_copy(vals[:, :, 0:D_OUT], vals_stage[:])
                nc.vector.tensor_copy(vals[:, :, D_OUT : D_OUT + 2], ones_kc[:])

            # ---------- per-tile x pipeline: cast + XBAR transpose ----------
            # Persistent double-buffered fp16 staging so the pad columns
            # (64..127, junk after transpose) are zeroed exactly once.
            xt16_all = constp.tile([128, 2, NSUB, 128], f16)
            nc.vector.memset(xt16_all[:, :, :, D_IN:128], 0.0)

            def assemble_xsT(xt32, i):
                slot = i % 2
                nc.vector.tensor_copy(xt16_all[:, slot, :, 0:D_IN], xt32[:])
                xsT = xsTp.tile([128, NSUB, 128], f16)
                # out[d, a, p] = in[p, a*128 + d]  (hardware XBAR mapping);
                # x lives in cols 0..63 of each 128 block -> rows 0..63.
                nc.sync.dma_start_transpose(
                    xsT[:], xt16_all[:, slot, :, :].rearrange("p a d -> p (a d)")
                )
                return xsT

            # ---------------- main loop ----------------
            def phase1_chunk(xsT2, E2, c):
                # same chunk of two row-tiles: shared stationary + shared
                # f32 c_sq bias on the single exp
                pe = psA.tile([128, 2, TROWS], f32, tag="psA")
                nc.tensor.matmul(pe[:, 0, :], lhs1[:, c, :], xsT2[0][0:D_IN, :, :])
                nc.tensor.matmul(pe[:, 1, :], lhs1[:, c, :], xsT2[1][0:D_IN, :, :])
                nc.scalar.activation(
                    E2[:, c, :, :], pe[:], Exp, scale=2.0,
                    bias=negcsq[:, c : c + 1],
                )

            def phase2_subtile(E2, t, gi, ysb, a):
                po = psO.tile([128, D_OUT + 2], f32, tag="psO")
                for c in range(KC):
                    nc.tensor.matmul(
                        po[:],
                        E2[:, c, t, a * 128 : (a + 1) * 128],
                        vals[:, c, :],
                        start=(c == 0),
                        stop=(c == KC - 1),
                    )
                rcp = rcpp.tile([128, 1], f32)
                nc.vector.reciprocal(rcp[:], po[:, D_OUT : D_OUT + 1])
                nc.vector.tensor_scalar_mul(ysb[:, a, :], po[:, 0:D_OUT], rcp[:])
                if a == NSUB - 1:
                    n0 = gi * TROWS
                    nc.gpsimd.dma_start(
                        y[n0 : n0 + TROWS, :].rearrange("(a p) v -> p a v", p=128),
                        ysb[:],
                    )

            xsT_cur = [
                assemble_xsT(xt_inflight[0], 0),
                assemble_xsT(xt_inflight[1], 1),
            ]
            Eprev = None
            ysb_pair = [None, None]
            for pi in range(npairs):
                if pi + 1 < npairs:
                    xt_inflight = [
                        issue_x_dma(2 * pi + 2),
                        issue_x_dma(2 * pi + 3),
                    ]
                E2 = Ep.tile([128, KC, 2, TROWS], bf16)
                xsT_next = None
                # 8 phase-1 chunk-pairs interleave 1:1 with the previous
                # pair's 8 phase-2 sub-tiles (engine queues are in-order:
                # the PE streams phase-2 while ACT drains the exp).
                for c in range(KC):
                    if pi == 0:
                        prep_chunk(c)
                        if c == 1:
                            stage_vals()
                    phase1_chunk(xsT_cur, E2, c)
                    if c == 2 and pi + 1 < npairs:
                        xsT_next = [
                            assemble_xsT(xt_inflight[0], 2 * pi + 2),
                            assemble_xsT(xt_inflight[1], 2 * pi + 3),
                        ]
                    if Eprev is not None:
                        t, a = divmod(c, NSUB)
                        if a == 0:
                            ysb_pair[t] = yp.tile(
                                [128, NSUB, D_OUT], f32, name="ysb"
                            )
                        phase2_subtile(
                            Eprev, t, 2 * (pi - 1) + t, ysb_pair[t], a
                        )
                Eprev = E2
                if xsT_next is not None:
                    xsT_cur = xsT_next
            for idx in range(2 * NSUB):
                t, a = divmod(idx, NSUB)
                if a == 0:
                    ysb_pair[t] = yp.tile([128, NSUB, D_OUT], f32, name="ysb")
                phase2_subtile(Eprev, t, 2 * (npairs - 1) + t, ysb_pair[t], a)

    nc.compile()
    nc.finalize()
    return nc


def get_nc(use_f32r=USE_F32R, rows=NS, dma="sync", ph2_bf16=True):
    key = ("nc", rows)
    if key not in _cache:
        _cache[key] = _build(rows)
    return _cache[key]


def make_in_maps(x, ctrs, values, s):
    x = np.ascontiguousarray(x, dtype=np.float32)
    ctrs = np.ascontiguousarray(ctrs, dtype=np.float32)
    values = np.ascontiguousarray(values, dtype=np.float32)
    s = np.ascontiguousarray(s, dtype=np.float32)
    return [
        {
            "x": x[i * NS : (i + 1) * NS],
            "ctrs": ctrs,
            "values": values,
            "s": s,
        }
        for i in range(NCORES)
    ]


def run(x, ctrs, values, s, trace=False, use_f32r=USE_F32R, tmpdir=None):
    from concourse.bass_utils import run_bass_kernel_spmd

    nc = get_nc(use_f32r)
    res = run_bass_kernel_spmd(
        nc,
        make_in_maps(x, ctrs, values, s),
        list(range(NCORES)),
        trace=trace,
        tmpdir=tmpdir,
    )
    out = np.concatenate([res.results[i]["y"] for i in range(NCORES)], axis=0)
    return out, res


def kernel(x, ctrs, values, s):
    out, _ = run(x, ctrs, values, s, trace=False)
    return out.astype(np.float32)


# revision 17
# speedup vs baseline: 1.2021x; 1.1750x over previous
"""Trainium2 Bass kernel for the vq_codebook problem.

  dist_sq[n,k] = sum_d (x[n,d]-ctrs[k,d])^2 * s[d]
  out = softmax(-dist_sq, axis=1) @ values

Sharding: data-parallel over N (8192 rows of x per core); ctrs/values/s
replicated on all 8 cores. No collectives (forward only).

Math trick: softmax is shift-invariant, so
  softmax(-dist_sq)[n,k] = softmax(2*cross_s[n,k] - c_sq[k])  with
  cross_s = (x*s) @ ctrs.T,  c_sq[k] = sum_d s[d]*ctrs[k,d]^2.
We compute E = exp(2*cross_s - c_sq) unnormalized (range-checked: max
exponent ~48 < 88, row-max min ~ -27, so fp32 exp never overflows and
denominators stay normal), then
  y[n,:] = (E.T @ values_aug)[n,:256] / (E.T @ values_aug)[n,256]
with values_aug = [values | ones] so the denominator comes from the same
accumulating matmul.

Layouts: phase 1 runs transposed (k on partitions, n on free): per
128-centroid chunk, stationary lhs1 = fp16(s*ctrs^T) [64, 128] against
the moving fp16 x^T [64, 512]. The -c_sq shift is applied as the exp
activation's per-partition f32 bias (partitions = k in this layout), so
it costs nothing on the PE and carries no fp16 quantization error. To
share one bias per activation, each exp covers the same chunk of TWO
row-tiles ([128, 2, 512]); the two matmuls of such a pair also share
the same stationary, halving weight loads. Phase-1 fp16 operands add
~4e-3 rel err vs the 2e-2 budget (validated numerically); fp16 gets PE
fast-weight-load so LDWEIGHTS hides under the previous matmul stream.

x^T and ctrs^T are produced by the DMA XBAR transpose engine (2-byte
dtypes), keeping the PE entirely free of transpose work. The XBAR
maps in[128, (blk 128)] -> out[d, blk, p] = in[p, blk*128 + d] (middle
output dim strides the source free index by 128 = the XBAR tile width
— verified against hardware), so x is staged in a padded [*, 128]
free layout with data in cols 0..63; the junk columns transpose into
partitions 64..127, which no access pattern ever reads.

Phase 2 uses bf16 E chunks as the stationary operand against
values_aug, producing y in natural [n, d_out] layout (fp8/DoubleRow was
evaluated numerically and busts the error budget; bf16 is the floor).

Pipelining: engine queues are in-order, so phase-1 chunk-pairs and
phase-2 sub-tiles are interleaved 1:1 in emission order — while the ACT
engine drains an exp (~1us), the PE streams the previous pair's phase-2
matmuls instead of stalling on the psA pool. The lhs1/c_sq prep (DVE +
ACT only) is likewise interleaved with pair-0 chunk by chunk.
"""

import os

os.environ.setdefault("JAX_PLATFORMS", "axon")

import numpy as np

N, D_IN, K, D_OUT = 65536, 64, 1024, 256
NCORES = 8
NS = N // NCORES  # 8192 rows per core
TROWS = 512  # rows of x per tile
NTILES = NS // TROWS  # 16
NPAIRS = NTILES // 2  # tiles are processed in pairs sharing exp bias
KC = K // 128  # 8 centroid chunks
NSUB = TROWS // 128  # 4 output sub-tiles per tile

USE_F32R = True

_cache = {}


def _build(rows=NS):
    import concourse.bacc as bacc
    import concourse.tile as tile
    from concourse import mybir

    f32 = mybir.dt.float32
    f16 = mybir.dt.float16
    bf16 = mybir.dt.bfloat16
    Exp = mybir.ActivationFunctionType.Exp
    Copy = mybir.ActivationFunctionType.Copy
    Mult = mybir.AluOpType.mult
    Add = mybir.AluOpType.add

    ntiles = rows // TROWS
    npairs = ntiles // 2
    nc = bacc.Bacc("TRN2", target_bir_lowering=False, debug=False)
    x = nc.declare_dram_parameter("x", [rows, D_IN], f32, isOutput=False)
    ctrs = nc.declare_dram_parameter("ctrs", [K, D_IN], f32, isOutput=False)
    values = nc.declare_dram_parameter("values", [K, D_OUT], f32, isOutput=False)
    s = nc.declare_dram_parameter("s", [D_IN], f32, isOutput=False)
    y = nc.declare_dram_parameter("y", [rows, D_OUT], f32, isOutput=True)

    with tile.TileContext(nc) as tc:
        with (
            tc.tile_pool(name="const", bufs=1) as constp,
            tc.tile_pool(name="tmp1", bufs=2) as tmp1p,
            tc.tile_pool(name="xt32", bufs=4) as xt32p,
            tc.tile_pool(name="xsT", bufs=4) as xsTp,
            tc.tile_pool(name="E", bufs=2) as Ep,
            tc.tile_pool(name="ysb", bufs=3) as yp,
            tc.tile_pool(name="rcp", bufs=8) as rcpp,
            tc.tile_pool(name="psA", bufs=3, space="PSUM") as psA,
            tc.tile_pool(name="psO", bufs=2, space="PSUM") as psO,
        ):
            # ---- startup DMAs first: tile-0/1 x, then the small constants --
            def issue_x_dma(i):
                n0 = i * TROWS
                xt32 = xt32p.tile([128, NSUB, D_IN], f32)
                nc.sync.dma_start(
                    xt32[:], x[n0 : n0 + TROWS, :].rearrange("(a p) d -> p a d", p=128)
                )
                return xt32

            xt_inflight = [issue_x_dma(0), issue_x_dma(1)]

            s_col = constp.tile([D_IN, 1], f32)
            nc.sync.dma_start(s_col[:], s[:].rearrange("(p o) -> p o", o=1))
            s_row = constp.tile([1, D_IN], f32)
            nc.sync.dma_start(s_row[:], s[:].rearrange("(o d) -> o d", o=1))
            ctrs32 = constp.tile([128, KC, D_IN], f32)
            nc.sync.dma_start(
                ctrs32[:], ctrs[:].rearrange("(c p) d -> p c d", p=128)
            )

            # s broadcast along partitions via a 1-deep outer-product matmul
            ones1 = constp.tile([1, 128], f32)
            nc.vector.memset(ones1[:], 1.0)
            sbc_ps = psO.tile([128, D_OUT + 2], f32, tag="psO")
            nc.tensor.matmul(sbc_ps[:, 0:D_IN], ones1[:], s_row[:])
            sbc = constp.tile([128, D_IN], f32)
            nc.vector.tensor_copy(sbc[:], sbc_ps[:, 0:D_IN])

            # ctrs^T via fp16 cast + XBAR transpose (padded free layout)
            ctrs16 = constp.tile([128, KC, 128], f16)
            nc.vector.memset(ctrs16[:, :, D_IN:128], 0.0)
            nc.vector.tensor_copy(ctrs16[:, :, 0:D_IN], ctrs32[:])
            ctrsT = constp.tile([128, KC, 128], f16)
            nc.sync.dma_start_transpose(
                ctrsT[:], ctrs16[:].rearrange("p c d -> p (c d)")
            )

            # lhs1[d, c, k'] = fp16(s[d] * ctrs[c*128+k', d]); negcsq[k', c]
            # = -sum_d s[d]*ctrs[c*128+k', d]^2 stays f32 (exp bias).
            lhs1 = constp.tile([D_IN, KC, 128], f16)
            negcsq = constp.tile([128, KC], f32)

            def prep_chunk(c):
                # DVE per-partition scalar multiply: keeps the ACT engine
                # pure-Exp (an ACT Copy here would thrash the 1.5us
                # activation table load between Copy and Exp sets)
                nc.vector.tensor_scalar_mul(
                    lhs1[:, c, :], ctrsT[0:D_IN, c, :], s_col[:]
                )
                t1 = tmp1p.tile([128, D_IN], f32)
                nc.vector.tensor_mul(t1[:], ctrs32[:, c, :], sbc[:])
                t2 = tmp1p.tile([128, D_IN], f32)
                nc.vector.tensor_mul(t2[:], t1[:], ctrs32[:, c, :])
                # (tensor_tensor_reduce w/ accum_out wedges the exec unit on
                # HW; tensor_reduce is fine)
                nc.vector.tensor_reduce(
                    negcsq[:, c : c + 1], t2[:],
                    axis=mybir.AxisListType.X, op=Add, negate=True,
                )

            for c in range(KC):
                prep_chunk(c)

            # values staging is only needed once phase 2 of pair 0 starts
            vals_stage = constp.tile([128, KC, D_OUT], f32)
            nc.gpsimd.dma_start(
                vals_stage[:], values[:].rearrange("(c p) v -> p c v", p=128)
            )
            ones_kc = constp.tile([128, KC, 2], f32)
            nc.vector.memset(ones_kc[:], 1.0)
            vals = constp.tile([128, KC, D_OUT + 2], bf16)
            nc.vector.tensor_copy(vals[:, :, 0:D_OUT], vals_stage[:])
            nc.vector.tensor_copy(vals[:, :, D_OUT : D_OUT + 2], ones_kc[:])

            # ---------- per-tile x pipeline: cast + XBAR transpose ----------
            # Persistent double-buffered fp16 staging so the pad columns
            # (64..127, junk after transpose) are zeroed exactly once.
            xt16_all = constp.tile([128, 2, NSUB, 128], f16)
            nc.vector.memset(xt16_all[:, :, :, D_IN:128], 0.0)

            def assemble_xsT(xt32, i):
                slot = i % 2
                nc.vector.tensor_copy(xt16_all[:, slot, :, 0:D_IN], xt32[:])
                xsT = xsTp.tile([128, NSUB, 128], f16)
                # out[d, a, p] = in[p, a*128 + d]  (hardware XBAR mapping);
                # x lives in cols 0..63 of each 128 block -> rows 0..63.
                nc.sync.dma_start_transpose(
                    xsT[:], xt16_all[:, slot, :, :].rearrange("p a d -> p (a d)")
                )
                return xsT

            # ---------------- main loop ----------------
            def phase1_chunk(xsT2, E2, c):
                # same chunk of two row-tiles: shared stationary + shared
                # f32 c_sq bias on the single exp
                pe = psA.tile([128, 2, TROWS], f32, tag="psA")
                nc.tensor.matmul(pe[:, 0, :], lhs1[:, c, :], xsT2[0][0:D_IN, :, :])
                nc.tensor.matmul(pe[:, 1, :], lhs1[:, c, :], xsT2[1][0:D_IN, :, :])
                nc.scalar.activation(
                    E2[:, c, :, :], pe[:], Exp, scale=2.0,
                    bias=negcsq[:, c : c + 1],
                )

            def phase2_subtile(E2, t, gi, ysb, a):
                po = psO.tile([128, D_OUT + 2], f32, tag="psO")
                for c in range(KC):
                    nc.tensor.matmul(
                        po[:],
                        E2[:, c, t, a * 128 : (a + 1) * 128],
                        vals[:, c, :],
                        start=(c == 0),
                        stop=(c == KC - 1),
                    )
                rcp = rcpp.tile([128, 1], f32)
                nc.vector.reciprocal(rcp[:], po[:, D_OUT : D_OUT + 1])
                nc.vector.tensor_scalar_mul(ysb[:, a, :], po[:, 0:D_OUT], rcp[:])
                if a == NSUB - 1:
                    n0 = gi * TROWS
                    nc.gpsimd.dma_start(
                        y[n0 : n0 + TROWS, :].rearrange("(a p) v -> p a v", p=128),
                        ysb[:],
                    )

            xsT_cur = [
                assemble_xsT(xt_inflight[0], 0),
                assemble_xsT(xt_inflight[1], 1),
            ]
            Eprev = None
            ysb_pair = [None, None]
            for pi in range(npairs):
                if pi + 1 < npairs:
                    xt_inflight = [
                        issue_x_dma(2 * pi + 2),
                        issue_x_dma(2 * pi + 3),
                    ]
                E2 = Ep.tile([128, KC, 2, TROWS], bf16)
                xsT_next = None
                # 8 phase-1 chunk-pairs interleave 1:1 with the previous
                # pair's 8 phase-2 sub-tiles (engine queues are in-order:
                # the PE streams phase-2 while ACT drains the exp).
                for c in range(KC):
                    phase1_chunk(xsT_cur, E2, c)
                    if c == 2 and pi + 1 < npairs:
                        xsT_next = [
                            assemble_xsT(xt_inflight[0], 2 * pi + 2),
                            assemble_xsT(xt_inflight[1], 2 * pi + 3),
                        ]
                    if Eprev is not None:
                        t, a = divmod(c, NSUB)
                        if a == 0:
                            ysb_pair[t] = yp.tile(
                                [128, NSUB, D_OUT], f32, name="ysb"
                            )
                        phase2_subtile(
                            Eprev, t, 2 * (pi - 1) + t, ysb_pair[t], a
                        )
                Eprev = E2
                if xsT_next is not None:
                    xsT_cur = xsT_next
            for idx in range(2 * NSUB):
                t, a = divmod(idx, NSUB)
                if a == 0:
                    ysb_pair[t] = yp.tile([128, NSUB, D_OUT], f32, name="ysb")
                phase2_subtile(Eprev, t, 2 * (npairs - 1) + t, ysb_pair[t], a)

    nc.compile()
    nc.finalize()
    return nc


def get_nc(use_f32r=USE_F32R, rows=NS, dma="sync", ph2_bf16=True):
    key = ("nc", rows)
    if key not in _cache:
        _cache[key] = _build(rows)
    return _cache[key]


def make_in_maps(x, ctrs, values, s):
    x = np.ascontiguousarray(x, dtype=np.float32)
    ctrs = np.ascontiguousarray(ctrs, dtype=np.float32)
    values = np.ascontiguousarray(values, dtype=np.float32)
    s = np.ascontiguousarray(s, dtype=np.float32)
    return [
        {
            "x": x[i * NS : (i + 1) * NS],
            "ctrs": ctrs,
            "values": values,
            "s": s,
        }
        for i in range(NCORES)
    ]


def run(x, ctrs, values, s, trace=False, use_f32r=USE_F32R, tmpdir=None):
    from concourse.bass_utils import run_bass_kernel_spmd

    nc = get_nc(use_f32r)
    res = run_bass_kernel_spmd(
        nc,
        make_in_maps(x, ctrs, values, s),
        list(range(NCORES)),
        trace=trace,
        tmpdir=tmpdir,
    )
    out = np.concatenate([res.results[i]["y"] for i in range(NCORES)], axis=0)
    return out, res


def kernel(x, ctrs, values, s):
    out, _ = run(x, ctrs, values, s, trace=False)
    return out.astype(np.float32)
